# revision 4
# baseline (speedup 1.0000x reference)
"""Differentiable A* forward pass on Trainium2 (Bass/Tile, 8-core data
parallel, 2 images/core stacked on the 128 SBUF partitions).

Device algorithm (bit-exact vs the JAX reference, host-validated):
 - state FM = f + BIG*(1-open): argmin(f | open) == reduce_min(FM)
 - cross-partition min via PE matmul "transpose" (exact: x*1.0 products,
   one nonzero term per output) + per-image reduce + PE broadcast
 - per-step scalars (rowsel/colsel/dsel) via row-accumulate + one PE
   matmul against a constant 128x128 image-block matrix
 - 3x3 neighbor mask from |row-rowsel|<=1 & |col-colsel|<=1 compares
 - state updates via exact masked blends (copy_predicated, u8 masks)
 - walrus here allows only ONE sync-wait per instruction: legalize_waits
   hoists extras onto same-engine NoOps.

Early exit: chunks of CHUNK steps; device reports per-image unsolved
flags; extra steps past global solve are output-neutral (validated).
Backtrack (int pointer chase) on host. Bit-exact host fallback retained.
"""
import numpy as np

B, SIZE = 16, 64
HW = SIZE * SIZE
NCORES = 8
STEPS_TOTAL = int(0.1 * HW)  # 409
CHUNK = 64
BIG = 1.0e9

_modules = {}

# pk input blocks (64 fp32 cols each unless noted)
BLKS = ["FM0", "H2", "DD", "OB", "ROWI", "COLI", "P64", "SM0", "HIST0",
        "PAR0", "A2a", "A2b", "MISC", "Wa", "Wb"]
NBLK = len(BLKS)
COL = {n: i * 64 for i, n in enumerate(BLKS)}
PO_COLS = 4 * 64 + 1


def _heur(goal, cost):
    Bn, H, W = goal.shape
    ii, jj = np.meshgrid(np.arange(H), np.arange(W), indexing="ij")
    loc = np.stack([ii, jj], 0).astype(np.float32)
    loc_e = loc.reshape(2, -1)[None]
    goal_loc = np.einsum("kij,bij->bk", loc, goal).astype(np.float32)
    d = np.abs(loc_e - goal_loc[:, :, None]).astype(np.float32)
    h = (d.sum(1, dtype=np.float32) - d.min(1)).astype(np.float32)
    euc = np.sqrt(((loc_e - goal_loc[:, :, None]) ** 2).sum(1, dtype=np.float32)
                  ).astype(np.float32)
    return (h + np.float32(0.001) * euc).astype(np.float32).reshape(Bn, H, W)


def _legalize_waits(nc, max_waits=1):
    import concourse.mybir as mybir
    total = 0
    for fn in nc.m.functions:
        for blk in fn.blocks:
            insts = list(blk.instructions)
            out = []
            changed = False
            for ins in insts:
                si = ins.sync_info
                waits = list(si.on_wait) if si and si.on_wait else []
                if len(waits) > max_waits:
                    for w in waits[:-max_waits]:
                        out.append(mybir.InstNoOp(
                            name=f"I-lw{nc.next_id()}",
                            sync_info=mybir.SyncInfo(on_wait=[w], on_update=[]),
                            bass_nofuse=True,
                            engine=ins.engine,
                        ))
                        total += 1
                    ins.sync_info = mybir.SyncInfo(
                        on_wait=waits[-max_waits:],
                        on_update=list(si.on_update) if si.on_update else [])
                    changed = True
                out.append(ins)
            if changed:
                try:
                    blk.instructions = out
                except Exception:
                    blk.instructions.clear()
                    for i in out:
                        blk.instructions.append(i)
    return total


def _build(steps):
    if steps in _modules:
        return _modules[steps]
    import concourse.bass as bass
    import concourse.mybir as mybir
    import concourse.tile as tile

    FP = mybir.dt.float32
    BF = mybir.dt.bfloat16
    U8 = mybir.dt.uint8
    ALU = mybir.AluOpType
    AX = mybir.AxisListType

    nc = bass.Bass()
    pk_d = nc.declare_dram_parameter("pk", [128, NBLK * 64], FP, isOutput=False)
    po_d = nc.declare_dram_parameter("po", [128, PO_COLS], FP, isOutput=True)

    with tile.TileContext(nc) as tc:
        with (
            tc.tile_pool(name="cst", bufs=1) as cst,
            tc.tile_pool(name="st", bufs=1) as st,
            tc.tile_pool(name="wk", bufs=2) as wk,
            tc.tile_pool(name="ps", bufs=2, space="PSUM") as ps,
        ):
            pkd = cst.tile([128, NBLK * 64], FP)
            nc.gpsimd.dma_start(pkd[:], pk_d[:])
            pk = cst.tile([128, NBLK * 64], FP)
            nc.vector.tensor_copy(pk[:], pkd[:])

            def c(name):
                return pk[:, COL[name]:COL[name] + 64]

            H2c, DDc, ROWIc, COLIc, P64c = (c(n) for n in
                                            ["H2", "DD", "ROWI", "COLI", "P64"])
            A2c = pk[0:2, COL["A2a"]:COL["A2a"] + 128]
            Wc = pk[:, COL["Wa"]:COL["Wa"] + 128]
            AMc = pk[:, COL["MISC"] + 2:COL["MISC"] + 4]
            ROWPc = pk[:, COL["MISC"]:COL["MISC"] + 1]
            GIDXc = pk[:, COL["MISC"] + 1:COL["MISC"] + 2]

            FM = st.tile([128, 64], FP)
            SM = st.tile([128, 64], BF)
            HIST = st.tile([128, 64], BF)
            PAR = st.tile([128, 64], FP)
            OBb = st.tile([128, 64], BF)
            UNSF = st.tile([128, 1], FP)
            nc.vector.tensor_copy(FM[:], c("FM0"))
            nc.vector.tensor_copy(SM[:], c("SM0"))
            nc.vector.tensor_copy(HIST[:], c("HIST0"))
            nc.vector.tensor_copy(PAR[:], c("PAR0"))
            nc.vector.tensor_copy(OBb[:], c("OB"))

            for i_ in range(steps):
                # ---- select: global argmin of FM per image ----
                rmin = wk.tile([128, 1], FP)
                nc.vector.tensor_reduce(out=rmin[:], in_=FM[:], axis=AX.X,
                                        op=ALU.min)
                lhs2 = wk.tile([128, 2], FP)
                nc.vector.tensor_scalar(out=lhs2[:], in0=AMc,
                                        scalar1=rmin[:, 0:1], scalar2=None,
                                        op0=ALU.mult)
                rT2 = ps.tile([2, 64], FP)
                nc.tensor.matmul(rT2[:], lhs2[:], P64c)
                gm2 = wk.tile([2, 1], FP)
                nc.vector.tensor_reduce(out=gm2[:], in_=rT2[0:2, :], axis=AX.X,
                                        op=ALU.min)
                bcg = ps.tile([128, 1], FP)
                nc.tensor.matmul(bcg[:], A2c, gm2[:])
                sel = wk.tile([128, 64], BF)
                nc.vector.tensor_scalar(out=sel[:], in0=FM[:],
                                        scalar1=bcg[:, 0:1], scalar2=None,
                                        op0=ALU.is_equal)
                nc.vector.tensor_tensor(out=HIST[:], in0=HIST[:], in1=sel[:],
                                        op=ALU.max)
                # ---- extract per-image scalars ----
                RS = wk.tile([128, 3], FP)
                junk = wk.tile([128, 64], FP)
                for j, other in enumerate((ROWIc, COLIc, DDc)):
                    nc.vector.scalar_tensor_tensor(
                        out=junk[:], in0=sel[:], scalar=1.0, in1=other,
                        op0=ALU.mult, op1=ALU.mult,
                        accum_out=RS[:, j:j + 1])
                dub = ps.tile([128, 3], FP)
                nc.tensor.matmul(dub[:], Wc, RS[:])
                SC = wk.tile([128, 5], FP)
                nc.vector.tensor_copy(SC[:, 0:3], dub[:, 0:3])
                # newp = 64*rowsel + colsel
                nc.vector.scalar_tensor_tensor(
                    out=SC[:, 3:4], in0=SC[:, 0:1], scalar=64.0,
                    op0=ALU.mult, in1=SC[:, 1:2], op1=ALU.add)
                SCu = wk.tile([128, 1], FP)
                nc.vector.tensor_scalar(out=SCu[:], in0=SC[:, 3:4],
                                        scalar1=GIDXc, scalar2=None,
                                        op0=ALU.not_equal)
                # v = gm + dsel ; w = H2 + v
                nc.vector.scalar_tensor_tensor(
                    out=SC[:, 4:5], in0=SC[:, 2:3], scalar=1.0,
                    op0=ALU.mult, in1=bcg[:, 0:1], op1=ALU.add)
                w = wk.tile([128, 64], FP)
                nc.vector.tensor_scalar(out=w[:], in0=H2c,
                                        scalar1=SC[:, 4:5], scalar2=None,
                                        op0=ALU.add)
                cmp = wk.tile([128, 64], BF)
                nc.vector.tensor_tensor(out=cmp[:], in0=FM[:], in1=w[:],
                                        op=ALU.is_gt)
                # ---- 3x3 neighborhood mask ----
                ca = wk.tile([128, 64], BF)
                cb = wk.tile([128, 64], BF)
                nc.vector.tensor_scalar(out=ca[:], in0=COLIc,
                                        scalar1=SC[:, 1:2], scalar2=1.0,
                                        op0=ALU.subtract, op1=ALU.is_le)
                nc.vector.tensor_scalar(out=cb[:], in0=COLIc,
                                        scalar1=SC[:, 1:2], scalar2=-1.0,
                                        op0=ALU.subtract, op1=ALU.is_ge)
                cm = wk.tile([128, 64], BF)
                nc.vector.tensor_tensor(out=cm[:], in0=ca[:], in1=cb[:],
                                        op=ALU.mult)
                ra = wk.tile([128, 1], FP)
                rb = wk.tile([128, 1], FP)
                nc.vector.tensor_scalar(out=ra[:], in0=ROWPc,
                                        scalar1=SC[:, 0:1], scalar2=1.0,
                                        op0=ALU.subtract, op1=ALU.is_le)
                nc.vector.tensor_scalar(out=rb[:], in0=ROWPc,
                                        scalar1=SC[:, 0:1], scalar2=-1.0,
                                        op0=ALU.subtract, op1=ALU.is_ge)
                rmk = wk.tile([128, 1], FP)
                nc.vector.tensor_tensor(out=rmk[:], in0=ra[:], in1=rb[:],
                                        op=ALU.mult)
                outer = wk.tile([128, 64], BF)
                nc.vector.tensor_scalar(out=outer[:], in0=cm[:],
                                        scalar1=rmk[:, 0:1], scalar2=None,
                                        op0=ALU.mult)
                ns = wk.tile([128, 64], BF)
                nc.vector.scalar_tensor_tensor(
                    out=ns[:], in0=sel[:], scalar=-1.0, op0=ALU.mult,
                    in1=outer[:], op1=ALU.add)
                nbr = wk.tile([128, 64], BF)
                nc.vector.tensor_tensor(out=nbr[:], in0=ns[:], in1=OBb[:],
                                        op=ALU.mult)
                # ---- updates ----
                rem = wk.tile([128, 64], BF)
                nc.vector.tensor_scalar(out=rem[:], in0=sel[:],
                                        scalar1=SCu[:, 0:1], scalar2=None,
                                        op0=ALU.mult)
                nc.vector.tensor_tensor(out=SM[:], in0=SM[:], in1=rem[:],
                                        op=ALU.subtract)
                q = wk.tile([128, 64], BF)
                nc.vector.tensor_scalar(out=q[:], in0=HIST[:], scalar1=-1.0,
                                        scalar2=1.0, op0=ALU.mult, op1=ALU.add)
                tq = wk.tile([128, 64], BF)
                nc.vector.tensor_tensor(out=tq[:], in0=cmp[:], in1=q[:],
                                        op=ALU.subtract)
                uq = wk.tile([128, 64], BF)
                nc.vector.tensor_tensor(out=uq[:], in0=SM[:], in1=tq[:],
                                        op=ALU.mult)
                i0 = wk.tile([128, 64], BF)
                nc.vector.tensor_tensor(out=i0[:], in0=q[:], in1=uq[:],
                                        op=ALU.add)
                idx = wk.tile([128, 64], BF)
                nc.vector.tensor_tensor(out=idx[:], in0=i0[:], in1=nbr[:],
                                        op=ALU.mult)
                idxu = wk.tile([128, 64], U8)
                nc.vector.tensor_scalar(out=idxu[:], in0=idx[:], scalar1=0.0,
                                        scalar2=None, op0=ALU.is_gt)
                # FM: close selected (if unsolved), then reopen idx at w
                nc.vector.scalar_tensor_tensor(
                    out=FM[:], in0=rem[:], scalar=BIG, op0=ALU.mult,
                    in1=FM[:], op1=ALU.add)
                nc.vector.copy_predicated(FM[:], idxu[:], w[:])
                nc.vector.copy_predicated(
                    PAR[:], idxu[:], SC[:, 3:4].broadcast_to([128, 64]))
                nc.vector.tensor_tensor(out=SM[:], in0=SM[:], in1=idx[:],
                                        op=ALU.max)
                if i_ == steps - 1:
                    nc.vector.tensor_copy(UNSF[:], SCu[:])

            po = st.tile([128, PO_COLS], FP)
            nc.vector.tensor_copy(po[:, 0:64], FM[:])
            nc.vector.tensor_copy(po[:, 64:128], SM[:])
            nc.vector.tensor_copy(po[:, 128:192], HIST[:])
            nc.vector.tensor_copy(po[:, 192:256], PAR[:])
            nc.vector.tensor_copy(po[:, 256:257], UNSF[:])
            nc.gpsimd.dma_start(po_d[:], po[:])

    _legalize_waits(nc)
    _modules[steps] = nc
    return nc


LAUNCH_NS = []
LAST_ERROR = None


def _device_solve(cost, start, goal, obst, H2, DD, goal_idx):
    import time
    from concourse.bass_utils import run_bass_kernel_spmd

    f32 = np.float32
    rowi = np.broadcast_to(np.arange(SIZE, dtype=f32)[:, None], (SIZE, SIZE))
    coli = np.broadcast_to(np.arange(SIZE, dtype=f32)[None, :], (SIZE, SIZE))
    iop = np.zeros((128, 64), f32)       # P64: partition -> its col
    for p in range(128):
        iop[p, p % 64] = 1.0
    a2 = np.zeros((128, 128), f32)       # rows 0:2 used
    a2[0, :64] = 1.0
    a2[1, 64:] = 1.0
    wmat = np.zeros((128, 128), f32)     # W[q,p] = img(q)==img(p)
    wmat[:64, :64] = 1.0
    wmat[64:, 64:] = 1.0
    misc = np.zeros((128, 64), f32)
    misc[:, 0] = np.tile(np.arange(SIZE, dtype=f32), 2)   # ROWP
    misc[:64, 2] = 1.0                                     # AM col 0
    misc[64:, 3] = 1.0                                     # AM col 1

    FMh = (H2 + f32(BIG) * (f32(1.0) - start)).astype(f32)
    SMh = start.astype(f32).copy()
    HISTh = np.zeros_like(start, dtype=f32)
    PARh = np.broadcast_to(goal_idx[:, None].astype(f32),
                           (B, HW)).reshape(B, SIZE, SIZE).copy()

    def stack2(a, ci):
        return np.concatenate([a[2 * ci], a[2 * ci + 1]], 0).astype(f32)

    done_steps = 0
    unsolved = np.ones(B, bool)
    while done_steps < STEPS_TOTAL and unsolved.any():
        steps = min(CHUNK, STEPS_TOTAL - done_steps)
        nc = _build(steps)
        in_maps = []
        for ci in range(NCORES):
            m = misc.copy()
            m[:64, 1] = goal_idx[2 * ci]
            m[64:, 1] = goal_idx[2 * ci + 1]
            blocks = [stack2(FMh, ci), stack2(H2, ci), stack2(DD, ci),
                      stack2(obst, ci),
                      np.concatenate([rowi, rowi], 0),
                      np.concatenate([coli, coli], 0), iop,
                      stack2(SMh, ci), stack2(HISTh, ci), stack2(PARh, ci),
                      a2[:, :64], a2[:, 64:], m, wmat[:, :64], wmat[:, 64:]]
            in_maps.append({"pk": np.concatenate(blocks, 1).astype(f32)})
        t0 = time.perf_counter()
        res = run_bass_kernel_spmd(nc, in_maps, core_ids=list(range(NCORES)))
        LAUNCH_NS.append((time.perf_counter() - t0) * 1e9)
        for ci in range(NCORES):
            r = res.results[ci]["po"]
            for arr, j in ((FMh, 0), (SMh, 1), (HISTh, 2), (PARh, 3)):
                arr[2 * ci] = r[:64, j * 64:(j + 1) * 64]
                arr[2 * ci + 1] = r[64:, j * 64:(j + 1) * 64]
            unsolved[2 * ci] = r[0, 256] > 0.5
            unsolved[2 * ci + 1] = r[64, 256] > 0.5
        done_steps += steps
    return HISTh, PARh


def _host_solve(cost, start, goal, obst, H2, DD, goal_idx):
    """Bit-exact numpy replica of the device algebra (and of the reference)."""
    f32 = np.float32
    COLI = np.broadcast_to(np.arange(SIZE, dtype=f32)[None, :], (SIZE, SIZE))
    ROWP = np.arange(SIZE, dtype=f32)[:, None]
    gidx = goal_idx.astype(f32)
    FM = (H2 + f32(BIG) * (f32(1.0) - start)).astype(f32)
    SM = start.astype(f32).copy()
    HIST = np.zeros_like(start, dtype=f32)
    PAR = np.broadcast_to(gidx[:, None], (B, HW)).astype(f32).reshape(
        B, SIZE, SIZE).copy()
    ROWI = np.broadcast_to(np.arange(SIZE, dtype=f32)[:, None], (SIZE, SIZE))
    unsolved = np.ones(B, bool)
    steps_done = 0
    while steps_done < STEPS_TOTAL and unsolved.any():
        chunk = min(CHUNK, STEPS_TOTAL - steps_done)
        for _ in range(chunk):
            gm = FM.reshape(B, -1).min(-1)
            sel = (FM == gm[:, None, None]).astype(f32)
            rowsel = (sel * ROWI).sum((1, 2), dtype=f32)
            colsel = (sel * COLI).sum((1, 2), dtype=f32)
            dsel = (sel * DD).sum((1, 2), dtype=f32)
            newp = (rowsel * f32(64) + colsel).astype(f32)
            uns = (newp != gidx).astype(f32)
            v = (gm + dsel).astype(f32)
            w = (H2 + v[:, None, None]).astype(f32)
            cm = (((COLI - colsel[:, None, None]) <= f32(1)) &
                  ((COLI - colsel[:, None, None]) >= f32(-1))).astype(f32)
            rm = (((ROWP[None] - rowsel[:, None, None]) <= f32(1)) &
                  ((ROWP[None] - rowsel[:, None, None]) >= f32(-1))).astype(f32)
            nbr = ((cm * rm - sel) * obst).astype(f32)
            rem = (sel * uns[:, None, None]).astype(f32)
            SM = (SM - rem).astype(f32)
            HIST = np.maximum(HIST, sel)
            cmp = (FM > w).astype(f32)
            q = (f32(1) - HIST).astype(f32)
            idx = ((q + SM * (cmp - q)) * nbr).astype(f32)
            FM = (FM + f32(BIG) * rem).astype(f32)
            FM = np.where(idx > 0, w, FM)
            PAR = np.where(idx > 0, newp[:, None, None], PAR)
            SM = np.maximum(SM, idx)
        steps_done += chunk
        unsolved &= uns > 0.5
    return HIST, PAR


def kernel(cost_maps, start_maps, goal_maps, obstacles_maps):
    f32 = np.float32
    cost = np.asarray(cost_maps, f32)[:, 0]
    start = np.asarray(start_maps, f32)[:, 0]
    goal = np.asarray(goal_maps, f32)[:, 0]
    obst = np.asarray(obstacles_maps, f32)[:, 0]
    heur = _heur(goal, cost)
    H2 = (f32(0.5) * (heur + cost).astype(f32)).astype(f32)
    DD = ((f32(0.5) * cost).astype(f32) - H2).astype(f32)
    goal_idx = goal.reshape(B, -1).argmax(-1)

    global LAST_ERROR
    try:
        HIST, PAR = _device_solve(cost, start, goal, obst, H2, DD, goal_idx)
    except Exception as e:
        import traceback
        LAST_ERROR = traceback.format_exc()
        HIST, PAR = _host_solve(cost, start, goal, obst, H2, DD, goal_idx)

    parents_i = PAR.reshape(B, HW).astype(np.int32)
    goal_flat = goal.reshape(B, -1).astype(np.int32)
    path = goal_flat.copy()
    loc = (parents_i * goal_flat).sum(-1)
    rows = np.arange(B)
    for _ in range(STEPS_TOTAL):
        path[rows, loc] = 1
        loc = parents_i[rows, loc]
    return (HIST[:, None].astype(np.float32),
            path.reshape(B, 1, SIZE, SIZE).astype(np.int32))
